# revision 22
# baseline (speedup 1.0000x reference)
"""DiscreteBipartiteFlow forward on 8 trn2 NeuronCores — v3.

Same math as v2 (see kernel_v2.py). Schedule changes driven by the v2 trace:
 - weights split into three DMAs (w1tb f32, w2h bf16, w2l+b2 bf16) and x into
   two half DMAs, ordered by need-time, so the NET matmul starts as soon as
   w2h lands instead of waiting for one big waux transfer.
 - relu chunks on DVE (idle then) + hh casts on ACT, pipelined per chunk, so
   the first hh.w2h matmul issues ~1.5us earlier.
 - argmax reads NET straight out of PSUM (no SBUF staging copy).
 - the pack-column broadcast transposes a stride-0 AP directly (no pkb
   materialization).
"""

import numpy as np
import ml_dtypes

V = 128
H = 512
N_CORES = 8
P = 128
NJ = 8
KH = H // P          # 4 contraction chunks, h = 4p + k


def build_bass(rows: int):
    import concourse.bacc as bacc
    import concourse.bass as bass
    import concourse.tile as tile
    from concourse import mybir

    f32 = mybir.dt.float32
    bf16 = mybir.dt.bfloat16
    i32 = mybir.dt.int32
    u32 = mybir.dt.uint32
    A = mybir.AluOpType
    ACT = mybir.ActivationFunctionType

    assert rows == P * NJ

    nc = bacc.Bacc(None)
    xbp = nc.declare_dram_parameter("xb", [rows, 2 * V], bf16, isOutput=False)
    w1tb = nc.declare_dram_parameter("w1tb", [P, KH, V + 1], f32, isOutput=False)
    # waux: [0:1024]=W2h rows 4p..4p+3, [1024:2048]=W2l, [2048:2304]: p0=b2h, p1=b2l
    waux = nc.declare_dram_parameter("waux", [P, 2 * KH * 2 * V + 2 * V], bf16, isOutput=False)
    zout = nc.declare_dram_parameter("zout", [rows, V], mybir.dt.int16, isOutput=True)

    x_r = xbp.rearrange("(p j) n -> p j n", j=NJ)
    zout_r = zout.rearrange("(p j) n -> p j n", j=NJ)

    def bcast_mid(t_ap, reps):
        return bass.AP(
            tensor=t_ap.tensor, offset=t_ap.offset,
            ap=[t_ap.ap[0], [0, reps]] + list(t_ap.ap[1:]),
        )

    def bcast_last(t_ap, reps):
        return bass.AP(
            tensor=t_ap.tensor, offset=t_ap.offset,
            ap=list(t_ap.ap) + [[0, reps]],
        )

    with tile.TileContext(nc) as tc:
        with (
            tc.tile_pool(name="main", bufs=1) as main,
            tc.tile_pool(name="scr", bufs=2) as scrp,
            tc.tile_pool(name="psum_net", bufs=1, space="PSUM") as psum_net,
            tc.tile_pool(name="psum_pb", bufs=1, space="PSUM") as psum_pb,
        ):
            # ---- DMA triggers (SP queue) in need order ----
            w1tb_sb = main.tile([P, KH, V + 1], f32)
            nc.sync.dma_start(out=w1tb_sb, in_=w1tb[:, :, :])
            waux_sb = main.tile([P, 2 * KH * 2 * V + 2 * V], bf16)
            nc.sync.dma_start(out=waux_sb, in_=waux[:, :])
            xt = main.tile([P, NJ, 2 * V], bf16)
            HJ = NJ // 2
            # h = relu(W1+b1) per chunk on DVE; hh casts on ACT (issued here,
            # before the x DMA, so the x-gate below can depend on relu).
            r = main.tile([P, KH, V], f32)
            hh = main.tile([P, KH, V], bf16)
            for k in range(KH):
                nc.vector.tensor_scalar(
                    out=r[:, k, :], in0=w1tb_sb[:, k, 0:V],
                    scalar1=w1tb_sb[:, k, V : V + 1], scalar2=0.0,
                    op0=A.add, op1=A.max,
                )
                nc.scalar.copy(out=hh[:, k, :], in_=r[:, k, :])
            # Delay the x load until the last relu chunk (~= all weights have
            # prime DMA bandwidth; x still lands well before the dots): junk
            # write into xt = WAW dep for the x DMA.
            nc.vector.tensor_copy(xt[0:1, 0:1, 0:1], r[0:1, KH - 1, 0:1])
            nc.sync.dma_start(out=xt, in_=x_r[:, :, :])

            # ---- device constants (no DMA deps) ----
            iota_i = main.tile([P, V], i32)
            nc.gpsimd.iota(iota_i, pattern=[[1, V]], channel_multiplier=0)
            iota_h = main.tile([P, V], mybir.dt.int16)
            nc.gpsimd.iota(iota_h, pattern=[[1, V]], channel_multiplier=0)
            ipart_i = main.tile([P, 1], i32)
            nc.gpsimd.iota(ipart_i, pattern=[[0, 1]], channel_multiplier=1)
            iota_f = main.tile([P, V], f32)
            nc.vector.tensor_copy(iota_f, iota_i)
            ipart_f = main.tile([P, 1], f32)
            nc.vector.tensor_copy(ipart_f, ipart_i)
            ident = main.tile([P, V], f32)
            nc.vector.tensor_tensor(
                out=ident, in0=iota_f, in1=bcast_last(ipart_f, V), op=A.is_equal
            )
            ones2 = main.tile([2, V], bf16)
            nc.vector.memset(ones2, 1.0)
            c127 = main.tile([P, 1], i32)
            nc.vector.memset(c127, 127)
            warm = main.tile([P, V], bf16)
            nc.gpsimd.memset(warm, 0.5)
            sel = main.tile([P, 2 * V], f32)
            # iota/128: the dot's inline x128 scalar restores it, so the
            # pack half lands at bit 7+ and a1 in bits 0-6.
            nc.vector.tensor_scalar(
                out=sel[:, V : 2 * V], in0=iota_f, scalar1=0.0078125,
                scalar2=None, op0=A.mult,
            )

            hl = main.tile([P, KH, V], bf16)
            for k in range(KH):
                nc.vector.scalar_tensor_tensor(
                    out=hl[:, k, :], in0=r[:, k, :], scalar=1.0, in1=hh[:, k, :],
                    op0=A.mult, op1=A.subtract,
                )

            # ---- PE p-state warmup: ~40 junk matmuls, no data deps ----
            warm_ps = psum_pb.tile([P, V], f32, tag="warm")
            for _ in range(40):
                nc.tensor.matmul(warm_ps, lhsT=warm, rhs=warm, start=True, stop=True)

            # ---- NET = hh@W2h + hh@W2l + hl@W2h + b2 (PE) ----
            net_ps = psum_net.tile([P, 2 * V], f32)
            w2h_sb = waux_sb[:, 0 : KH * 2 * V].rearrange("p (k n) -> p k n", k=KH)
            w2l_v = waux_sb[:, KH * 2 * V : 2 * KH * 2 * V].rearrange("p (k n) -> p k n", k=KH)
            for k in range(KH):
                nc.tensor.matmul(
                    net_ps, lhsT=hh[:, k, :], rhs=w2h_sb[:, k, :],
                    start=(k == 0), stop=False,
                )
            for k in range(KH):
                nc.tensor.matmul(
                    net_ps, lhsT=hh[:, k, :], rhs=w2l_v[:, k, :],
                    start=False, stop=False,
                )
            nc.tensor.matmul(
                net_ps, lhsT=ones2, rhs=waux_sb[0:2, 2 * KH * 2 * V :],
                start=False, stop=False,
            )
            for k in range(KH):
                nc.tensor.matmul(
                    net_ps, lhsT=hl[:, k, :], rhs=w2h_sb[:, k, :],
                    start=False, stop=(k == KH - 1),
                )

            # ---- argmax heads straight from PSUM + pack (DVE) ----
            # independent maxes first so the DVE queue pipelines them
            m8 = main.tile([P, 8], f32, tag="m8L")
            nc.vector.max(m8, net_ps[:, 0:V])
            m8s = main.tile([P, 8], f32, tag="m8S")
            nc.vector.max(m8s, net_ps[:, V : 2 * V])
            ixL = main.tile([P, 8], u32)
            nc.vector.max_index(ixL, m8, net_ps[:, 0:V])
            ixS = main.tile([P, 8], u32)
            nc.vector.max_index(ixS, m8s, net_ps[:, V : 2 * V])

            lf = main.tile([P, 1], f32)
            nc.vector.tensor_copy(lf, ixL[:, 0:1])
            sf = main.tile([P, 1], f32)
            nc.vector.tensor_copy(sf, ixS[:, 0:1])
            zf = main.tile([P, 1], f32)
            nc.vector.tensor_scalar(out=zf, in0=sf, scalar1=0.5, scalar2=None, op0=A.is_lt)
            pk0 = main.tile([P, 1], f32)
            nc.vector.tensor_scalar(
                out=pk0, in0=sf, scalar1=128.0, scalar2=lf[:, 0:1],
                op0=A.mult, op1=A.add,
            )
            pkf = main.tile([P, 1], f32)
            nc.vector.tensor_scalar(
                out=pkf, in0=zf, scalar1=16384.0, scalar2=pk0[:, 0:1],
                op0=A.mult, op1=A.add,
            )

            # ---- broadcast pack column via stride-0 transpose ----
            pb_ps = psum_pb.tile([P, V], f32)
            nc.tensor.transpose(pb_ps, bcast_last(pkf[:, 0:1], V), ident)
            nc.vector.tensor_copy(sel[:, 0:V], pb_ps)

            # ---- per-slot fused dot: comb = pack[i0] + a1 ----
            comb = main.tile([P, NJ], f32)
            for j in range(NJ):
                scr = scrp.tile([P, 2 * V], f32, tag=f"s{j % 2}")
                nc.vector.scalar_tensor_tensor(
                    out=scr, in0=xt[:, j, :], scalar=128.0, in1=sel,
                    op0=A.mult, op1=A.mult, accum_out=comb[:, j : j + 1],
                )

            # ---- unpack comb -> compare index ----
            ci = main.tile([P, NJ], i32)
            nc.vector.tensor_copy(ci, comb)
            a1i = main.tile([P, NJ], i32)
            nc.vector.tensor_scalar(out=a1i, in0=ci, scalar1=127, scalar2=None, op0=A.bitwise_and)
            s14 = main.tile([P, NJ], i32)
            nc.vector.tensor_scalar(out=s14, in0=ci, scalar1=14, scalar2=None, op0=A.arith_shift_right)
            s7 = main.tile([P, NJ], i32)
            nc.vector.tensor_scalar(out=s7, in0=ci, scalar1=7, scalar2=None, op0=A.arith_shift_right)
            z8 = main.tile([P, NJ], i32)
            nc.vector.tensor_scalar(
                out=z8, in0=ci, scalar1=21, scalar2=8,
                op0=A.arith_shift_right, op1=A.arith_shift_left,
            )
            ti = main.tile([P, NJ], i32)
            nc.vector.tensor_mul(ti, s14, a1i)
            t2 = main.tile([P, NJ], i32)
            nc.vector.tensor_add(t2, s7, ti)
            cr = main.tile([P, NJ], i32)
            nc.vector.scalar_tensor_tensor(
                out=cr, in0=t2, scalar=c127[:, 0:1], in1=z8,
                op0=A.bitwise_and, op1=A.bitwise_or,
            )
            cr16 = main.tile([P, NJ], mybir.dt.int16)
            nc.vector.tensor_copy(cr16, cr)


            # ---- z1 in its own tile; stream only z1 halves ----
            z1t = main.tile([P, NJ, V], mybir.dt.int16)
            for h in range(2):
                js = h * HJ
                nc.vector.tensor_tensor(
                    out=z1t[:, js : js + HJ, :],
                    in0=bcast_mid(iota_h, HJ),
                    in1=bcast_last(cr16[:, js : js + HJ], V),
                    op=A.is_equal,
                )
                nc.sync.dma_start(
                    out=zout_r[:, js : js + HJ, :],
                    in_=z1t[:, js : js + HJ, :],
                )

    nc.finalize()
    return nc


def _split_bf16(a: np.ndarray):
    hi = a.astype(ml_dtypes.bfloat16)
    lo = (a - hi.astype(np.float32)).astype(ml_dtypes.bfloat16)
    return hi, lo


def _host_marshal(W1, b1, W2, b2):
    w1tb = np.empty((P, KH, V + 1), np.float32)
    w1tb[:, :, :V] = W1.T.reshape(P, KH, V)
    w1tb[:, :, V] = b1.reshape(P, KH)
    w2hv, w2lv = _split_bf16(W2.astype(np.float32))
    b2h, b2l = _split_bf16(b2.astype(np.float32))
    waux = np.zeros((P, 2 * KH * 2 * V + 2 * V), ml_dtypes.bfloat16)
    waux[:, 0 : KH * 2 * V] = w2hv.reshape(P, KH * 2 * V)
    waux[:, KH * 2 * V : 2 * KH * 2 * V] = w2lv.reshape(P, KH * 2 * V)
    waux[0, 2 * KH * 2 * V :] = b2h
    waux[1, 2 * KH * 2 * V :] = b2l
    return np.ascontiguousarray(w1tb), np.ascontiguousarray(waux)


RUN_KWARGS: dict = {}
LAST_RESULTS = None


def kernel(**inputs) -> np.ndarray:
    global LAST_RESULTS
    from concourse.bass_utils import run_bass_kernel_spmd

    x = np.ascontiguousarray(np.asarray(inputs["inputs"], dtype=np.float32))
    W1 = np.asarray(inputs["W1"], dtype=np.float32)
    b1 = np.asarray(inputs["b1"], dtype=np.float32)
    W2 = np.asarray(inputs["W2"], dtype=np.float32)
    b2 = np.asarray(inputs["b2"], dtype=np.float32)
    w1tb, waux = _host_marshal(W1, b1, W2, b2)

    B = x.shape[0]
    rows = B // N_CORES
    nc = build_bass(rows)

    xb = x.astype(ml_dtypes.bfloat16)
    bshards = np.split(xb, N_CORES, axis=0)
    in_maps = [{"xb": s, "w1tb": w1tb, "waux": waux} for s in bshards]
    res = run_bass_kernel_spmd(nc, in_maps, list(range(N_CORES)), **RUN_KWARGS)
    LAST_RESULTS = res
    z1 = np.concatenate([r["zout"] for r in res.results], axis=0).astype(np.float32)
    return np.ascontiguousarray(np.concatenate([x[:, :V], z1], axis=1))


# revision 23
# speedup vs baseline: 1.0947x; 1.0947x over previous
"""DiscreteBipartiteFlow forward on 8 trn2 NeuronCores — v3.

Same math as v2 (see kernel_v2.py). Schedule changes driven by the v2 trace:
 - weights split into three DMAs (w1tb f32, w2h bf16, w2l+b2 bf16) and x into
   two half DMAs, ordered by need-time, so the NET matmul starts as soon as
   w2h lands instead of waiting for one big waux transfer.
 - relu chunks on DVE (idle then) + hh casts on ACT, pipelined per chunk, so
   the first hh.w2h matmul issues ~1.5us earlier.
 - argmax reads NET straight out of PSUM (no SBUF staging copy).
 - the pack-column broadcast transposes a stride-0 AP directly (no pkb
   materialization).
"""

import numpy as np
import ml_dtypes

V = 128
H = 512
N_CORES = 8
P = 128
NJ = 8
KH = H // P          # 4 contraction chunks, h = 4p + k


def build_bass(rows: int):
    import concourse.bacc as bacc
    import concourse.bass as bass
    import concourse.tile as tile
    from concourse import mybir

    f32 = mybir.dt.float32
    bf16 = mybir.dt.bfloat16
    i32 = mybir.dt.int32
    u32 = mybir.dt.uint32
    A = mybir.AluOpType
    ACT = mybir.ActivationFunctionType

    assert rows == P * NJ

    nc = bacc.Bacc(None)
    xbp = nc.declare_dram_parameter("xb", [rows, 2 * V], bf16, isOutput=False)
    w1tb = nc.declare_dram_parameter("w1tb", [P, KH, V + 1], f32, isOutput=False)
    # waux: [0:1024]=W2h rows 4p..4p+3, [1024:2048]=W2l, [2048:2304]: p0=b2h, p1=b2l
    waux = nc.declare_dram_parameter("waux", [P, 2 * KH * 2 * V + 2 * V], bf16, isOutput=False)
    zout = nc.declare_dram_parameter("zout", [rows, V], bf16, isOutput=True)

    x_r = xbp.rearrange("(p j) n -> p j n", j=NJ)
    zout_r = zout.rearrange("(p j) n -> p j n", j=NJ)

    def bcast_mid(t_ap, reps):
        return bass.AP(
            tensor=t_ap.tensor, offset=t_ap.offset,
            ap=[t_ap.ap[0], [0, reps]] + list(t_ap.ap[1:]),
        )

    def bcast_last(t_ap, reps):
        return bass.AP(
            tensor=t_ap.tensor, offset=t_ap.offset,
            ap=list(t_ap.ap) + [[0, reps]],
        )

    with tile.TileContext(nc) as tc:
        with (
            tc.tile_pool(name="main", bufs=1) as main,
            tc.tile_pool(name="scr", bufs=2) as scrp,
            tc.tile_pool(name="psum_net", bufs=1, space="PSUM") as psum_net,
            tc.tile_pool(name="psum_pb", bufs=1, space="PSUM") as psum_pb,
        ):
            # ---- DMA triggers (SP queue) in need order ----
            w1tb_sb = main.tile([P, KH, V + 1], f32)
            nc.sync.dma_start(out=w1tb_sb, in_=w1tb[:, :, :])
            waux_sb = main.tile([P, 2 * KH * 2 * V + 2 * V], bf16)
            nc.sync.dma_start(out=waux_sb, in_=waux[:, :])
            xt = main.tile([P, NJ, 2 * V], bf16)
            HJ = NJ // 2
            # h = relu(W1+b1) per chunk on DVE; hh casts on ACT (issued here,
            # before the x DMA, so the x-gate below can depend on relu).
            r = main.tile([P, KH, V], f32)
            hh = main.tile([P, KH, V], bf16)
            for k in range(KH):
                nc.vector.tensor_scalar(
                    out=r[:, k, :], in0=w1tb_sb[:, k, 0:V],
                    scalar1=w1tb_sb[:, k, V : V + 1], scalar2=0.0,
                    op0=A.add, op1=A.max,
                )
                nc.scalar.copy(out=hh[:, k, :], in_=r[:, k, :])
            # Delay the x load until the last relu chunk (~= all weights have
            # prime DMA bandwidth; x still lands well before the dots): junk
            # write into xt = WAW dep for the x DMA.
            nc.vector.tensor_copy(xt[0:1, 0:1, 0:1], r[0:1, KH - 1, 0:1])
            nc.sync.dma_start(out=xt, in_=x_r[:, :, :])

            # ---- device constants (no DMA deps) ----
            iota_i = main.tile([P, V], i32)
            nc.gpsimd.iota(iota_i, pattern=[[1, V]], channel_multiplier=0)
            iota_h = main.tile([P, V], mybir.dt.int16)
            nc.gpsimd.iota(iota_h, pattern=[[1, V]], channel_multiplier=0)
            ipart_i = main.tile([P, 1], i32)
            nc.gpsimd.iota(ipart_i, pattern=[[0, 1]], channel_multiplier=1)
            iota_f = main.tile([P, V], f32)
            nc.vector.tensor_copy(iota_f, iota_i)
            ipart_f = main.tile([P, 1], f32)
            nc.vector.tensor_copy(ipart_f, ipart_i)
            ident = main.tile([P, V], f32)
            nc.vector.tensor_tensor(
                out=ident, in0=iota_f, in1=bcast_last(ipart_f, V), op=A.is_equal
            )
            ones2 = main.tile([2, V], bf16)
            nc.vector.memset(ones2, 1.0)
            c127 = main.tile([P, 1], i32)
            nc.vector.memset(c127, 127)
            warm = main.tile([P, V], bf16)
            nc.gpsimd.memset(warm, 0.5)
            sel = main.tile([P, 2 * V], f32)
            # iota/128: the dot's inline x128 scalar restores it, so the
            # pack half lands at bit 7+ and a1 in bits 0-6.
            nc.vector.tensor_scalar(
                out=sel[:, V : 2 * V], in0=iota_f, scalar1=0.0078125,
                scalar2=None, op0=A.mult,
            )

            hl = main.tile([P, KH, V], bf16)
            for k in range(KH):
                nc.vector.scalar_tensor_tensor(
                    out=hl[:, k, :], in0=r[:, k, :], scalar=1.0, in1=hh[:, k, :],
                    op0=A.mult, op1=A.subtract,
                )

            # ---- PE p-state warmup: ~40 junk matmuls, no data deps ----
            warm_ps = psum_pb.tile([P, V], f32, tag="warm")
            for _ in range(40):
                nc.tensor.matmul(warm_ps, lhsT=warm, rhs=warm, start=True, stop=True)

            # ---- NET = hh@W2h + hh@W2l + hl@W2h + b2 (PE) ----
            net_ps = psum_net.tile([P, 2 * V], f32)
            w2h_sb = waux_sb[:, 0 : KH * 2 * V].rearrange("p (k n) -> p k n", k=KH)
            w2l_v = waux_sb[:, KH * 2 * V : 2 * KH * 2 * V].rearrange("p (k n) -> p k n", k=KH)
            for k in range(KH):
                nc.tensor.matmul(
                    net_ps, lhsT=hh[:, k, :], rhs=w2h_sb[:, k, :],
                    start=(k == 0), stop=False,
                )
            for k in range(KH):
                nc.tensor.matmul(
                    net_ps, lhsT=hh[:, k, :], rhs=w2l_v[:, k, :],
                    start=False, stop=False,
                )
            nc.tensor.matmul(
                net_ps, lhsT=ones2, rhs=waux_sb[0:2, 2 * KH * 2 * V :],
                start=False, stop=False,
            )
            for k in range(KH):
                nc.tensor.matmul(
                    net_ps, lhsT=hl[:, k, :], rhs=w2h_sb[:, k, :],
                    start=False, stop=(k == KH - 1),
                )

            # ---- argmax heads straight from PSUM + pack (DVE) ----
            # independent maxes first so the DVE queue pipelines them
            m8 = main.tile([P, 8], f32, tag="m8L")
            nc.vector.max(m8, net_ps[:, 0:V])
            m8s = main.tile([P, 8], f32, tag="m8S")
            nc.vector.max(m8s, net_ps[:, V : 2 * V])
            ixL = main.tile([P, 8], u32)
            nc.vector.max_index(ixL, m8, net_ps[:, 0:V])
            ixS = main.tile([P, 8], u32)
            nc.vector.max_index(ixS, m8s, net_ps[:, V : 2 * V])

            lf = main.tile([P, 1], f32)
            nc.vector.tensor_copy(lf, ixL[:, 0:1])
            sf = main.tile([P, 1], f32)
            nc.vector.tensor_copy(sf, ixS[:, 0:1])
            zf = main.tile([P, 1], f32)
            nc.vector.tensor_scalar(out=zf, in0=sf, scalar1=0.5, scalar2=None, op0=A.is_lt)
            pk0 = main.tile([P, 1], f32)
            nc.vector.tensor_scalar(
                out=pk0, in0=sf, scalar1=128.0, scalar2=lf[:, 0:1],
                op0=A.mult, op1=A.add,
            )
            pkf = main.tile([P, 1], f32)
            nc.vector.tensor_scalar(
                out=pkf, in0=zf, scalar1=16384.0, scalar2=pk0[:, 0:1],
                op0=A.mult, op1=A.add,
            )

            # ---- broadcast pack column via stride-0 transpose ----
            pb_ps = psum_pb.tile([P, V], f32)
            nc.tensor.transpose(pb_ps, bcast_last(pkf[:, 0:1], V), ident)
            nc.vector.tensor_copy(sel[:, 0:V], pb_ps)

            # ---- per-slot fused dot: comb = pack[i0] + a1 ----
            comb = main.tile([P, NJ], f32)
            for j in range(NJ):
                scr = scrp.tile([P, 2 * V], f32, tag=f"s{j % 2}")
                nc.vector.scalar_tensor_tensor(
                    out=scr, in0=xt[:, j, :], scalar=128.0, in1=sel,
                    op0=A.mult, op1=A.mult, accum_out=comb[:, j : j + 1],
                )

            # ---- unpack comb -> compare index ----
            ci = main.tile([P, NJ], i32)
            nc.vector.tensor_copy(ci, comb)
            a1i = main.tile([P, NJ], i32)
            nc.vector.tensor_scalar(out=a1i, in0=ci, scalar1=127, scalar2=None, op0=A.bitwise_and)
            s14 = main.tile([P, NJ], i32)
            nc.vector.tensor_scalar(out=s14, in0=ci, scalar1=14, scalar2=None, op0=A.arith_shift_right)
            s7 = main.tile([P, NJ], i32)
            nc.vector.tensor_scalar(out=s7, in0=ci, scalar1=7, scalar2=None, op0=A.arith_shift_right)
            z8 = main.tile([P, NJ], i32)
            nc.vector.tensor_scalar(
                out=z8, in0=ci, scalar1=21, scalar2=8,
                op0=A.arith_shift_right, op1=A.arith_shift_left,
            )
            ti = main.tile([P, NJ], i32)
            nc.vector.tensor_mul(ti, s14, a1i)
            t2 = main.tile([P, NJ], i32)
            nc.vector.tensor_add(t2, s7, ti)
            cr = main.tile([P, NJ], i32)
            nc.vector.scalar_tensor_tensor(
                out=cr, in0=t2, scalar=c127[:, 0:1], in1=z8,
                op0=A.bitwise_and, op1=A.bitwise_or,
            )
            cr16 = main.tile([P, NJ], mybir.dt.int16)
            nc.vector.tensor_copy(cr16, cr)


            # ---- z1 in its own tile; stream only z1 halves ----
            z1t = main.tile([P, NJ, V], bf16)
            for h in range(2):
                js = h * HJ
                nc.vector.tensor_tensor(
                    out=z1t[:, js : js + HJ, :],
                    in0=bcast_mid(iota_h, HJ),
                    in1=bcast_last(cr16[:, js : js + HJ], V),
                    op=A.is_equal,
                )
                nc.sync.dma_start(
                    out=zout_r[:, js : js + HJ, :],
                    in_=z1t[:, js : js + HJ, :],
                )

    nc.finalize()
    return nc


def _split_bf16(a: np.ndarray):
    hi = a.astype(ml_dtypes.bfloat16)
    lo = (a - hi.astype(np.float32)).astype(ml_dtypes.bfloat16)
    return hi, lo


def _host_marshal(W1, b1, W2, b2):
    w1tb = np.empty((P, KH, V + 1), np.float32)
    w1tb[:, :, :V] = W1.T.reshape(P, KH, V)
    w1tb[:, :, V] = b1.reshape(P, KH)
    w2hv, w2lv = _split_bf16(W2.astype(np.float32))
    b2h, b2l = _split_bf16(b2.astype(np.float32))
    waux = np.zeros((P, 2 * KH * 2 * V + 2 * V), ml_dtypes.bfloat16)
    waux[:, 0 : KH * 2 * V] = w2hv.reshape(P, KH * 2 * V)
    waux[:, KH * 2 * V : 2 * KH * 2 * V] = w2lv.reshape(P, KH * 2 * V)
    waux[0, 2 * KH * 2 * V :] = b2h
    waux[1, 2 * KH * 2 * V :] = b2l
    return np.ascontiguousarray(w1tb), np.ascontiguousarray(waux)


RUN_KWARGS: dict = {}
LAST_RESULTS = None


def kernel(**inputs) -> np.ndarray:
    global LAST_RESULTS
    from concourse.bass_utils import run_bass_kernel_spmd

    x = np.ascontiguousarray(np.asarray(inputs["inputs"], dtype=np.float32))
    W1 = np.asarray(inputs["W1"], dtype=np.float32)
    b1 = np.asarray(inputs["b1"], dtype=np.float32)
    W2 = np.asarray(inputs["W2"], dtype=np.float32)
    b2 = np.asarray(inputs["b2"], dtype=np.float32)
    w1tb, waux = _host_marshal(W1, b1, W2, b2)

    B = x.shape[0]
    rows = B // N_CORES
    nc = build_bass(rows)

    xb = x.astype(ml_dtypes.bfloat16)
    bshards = np.split(xb, N_CORES, axis=0)
    in_maps = [{"xb": s, "w1tb": w1tb, "waux": waux} for s in bshards]
    res = run_bass_kernel_spmd(nc, in_maps, list(range(N_CORES)), **RUN_KWARGS)
    LAST_RESULTS = res
    z1 = np.concatenate([r["zout"] for r in res.results], axis=0).astype(np.float32)
    return np.ascontiguousarray(np.concatenate([x[:, :V], z1], axis=1))
